# revision 1
# baseline (speedup 1.0000x reference)
"""Trainium2 Bass kernel: e3nn edge message block (gnn_message_passing).

Strategy (edge-parallel across 8 cores):
  - Host: fold norm constants into weights, build feature-major layouts,
    shard edges 25000/core (padded to 49*512).
  - Device phase A: node tables  s_T/vx_T/vy_T/vz_T = linear_up(node_feats).T
    built with float32r matmuls, stored bf16 interleaved in SBUF:
        T1[u, n, :] = (s, vx),  T2[u, n, :] = (vy, vz)
  - Device phase B, per 512-edge tile (feature-major [128, 512] working set):
      * gpsimd.ap_gather pulls per-edge sender rows from the SBUF tables
      * radial MLP on PE (float32r), silu on ACT
      * uvu tensor product as elementwise DVE ops + 8 accumulating matmuls
      * output written feature-major [512, E]; host transposes back
"""

import os
import sys

sys.path.insert(0, "/opt/trn_rl_repo")

import numpy as np

MUL = 128
N_NODES = 10000
N_EDGES = 200000
N_CORES = 8
ES = N_EDGES // N_CORES          # 25000 edges per core
F = 512                          # edges per tile (free dim)
NT = (ES + F - 1) // F           # 49 tiles
ESP = NT * F                     # 25088 padded edges per core
EDGE_FEAT_DIM = 8
HIDDEN = 64


def _silu_cst():
    z = np.linspace(-12.0, 12.0, 200001)
    pdf = np.exp(-0.5 * z * z) / np.sqrt(2.0 * np.pi)
    silu = z / (1.0 + np.exp(-z))
    trapz = getattr(np, "trapezoid", None) or getattr(np, "trapz")
    return np.float32(1.0 / np.sqrt(trapz(silu * silu * pdf, z)))


def build_program(n_nodes=N_NODES, f=F, nt=NT):
    """Build the SPMD single-core Bass program (same program on all cores)."""
    import concourse.bass as bass
    import concourse.bacc as bacc
    import concourse.tile as tile
    from concourse import mybir

    f32 = mybir.dt.float32
    f32r = mybir.dt.float32r
    bf16 = mybir.dt.bfloat16
    i16 = mybir.dt.int16
    AF = mybir.ActivationFunctionType

    esp = nt * f
    nc = bacc.Bacc(None, target_bir_lowering=False, debug=False)

    # ---- DRAM parameters --------------------------------------------------
    nfT = nc.declare_dram_parameter("nfT", [4 * MUL, n_nodes], bf16, isOutput=False)
    idx_d = nc.declare_dram_parameter("idx", [128, nt * (f // 16)], i16, isOutput=False)
    efT_d = nc.declare_dram_parameter("efT", [EDGE_FEAT_DIM, esp], bf16, isOutput=False)
    yT_d = nc.declare_dram_parameter("yT", [1, 4 * esp], bf16, isOutput=False)
    W1_d = nc.declare_dram_parameter("W1", [EDGE_FEAT_DIM, HIDDEN], bf16, isOutput=False)
    W2_d = nc.declare_dram_parameter("W2", [HIDDEN, HIDDEN], bf16, isOutput=False)
    W3_d = nc.declare_dram_parameter("W3", [HIDDEN, HIDDEN], bf16, isOutput=False)
    W4_d = nc.declare_dram_parameter("W4", [HIDDEN, 4 * MUL], bf16, isOutput=False)
    WupS_d = nc.declare_dram_parameter("WupS", [MUL, MUL], bf16, isOutput=False)
    WupV_d = nc.declare_dram_parameter("WupV", [MUL, MUL], bf16, isOutput=False)
    Wout_d = nc.declare_dram_parameter("Wout", [MUL, 4 * MUL], bf16, isOutput=False)
    ones_d = nc.declare_dram_parameter("ones", [1, MUL], bf16, isOutput=False)
    outT_d = nc.declare_dram_parameter("outT", [4 * MUL, esp], f32, isOutput=True)

    with tile.TileContext(nc) as tc:
        with (
            tc.tile_pool(name="const", bufs=1) as const,
            tc.tile_pool(name="tables", bufs=1) as tabs,
            tc.tile_pool(name="work", bufs=2) as work,
            tc.tile_pool(name="psum", bufs=2, space="PSUM") as psum,
        ):
            # ---- constants into SBUF -------------------------------------
            def cload(dram, shape, dtype, name):
                t = const.tile(shape, dtype, name=name, tag=name)
                nc.sync.dma_start(out=t[:], in_=dram[:])
                return t

            W1_s = cload(W1_d, [EDGE_FEAT_DIM, HIDDEN], bf16, "cW1")
            W2_s = cload(W2_d, [HIDDEN, HIDDEN], bf16, "cW2")
            W3_s = cload(W3_d, [HIDDEN, HIDDEN], bf16, "cW3")
            W4_s = cload(W4_d, [HIDDEN, 4 * MUL], bf16, "cW4")
            WupS_s = cload(WupS_d, [MUL, MUL], bf16, "cWupS")
            WupV_s = cload(WupV_d, [MUL, MUL], bf16, "cWupV")
            Wout_s = cload(Wout_d, [MUL, 4 * MUL], bf16, "cWout")  # A|B|C|D blocks
            ones_s = cload(ones_d, [1, MUL], bf16, "cones")
            idx_s = const.tile([128, nt * (f // 16)], i16, name="cidx", tag="cidx")
            nc.sync.dma_start(out=idx_s[:], in_=idx_d[:])

            A_s = Wout_s[:, 0:MUL]
            B_s = Wout_s[:, MUL : 2 * MUL]
            C_s = Wout_s[:, 2 * MUL : 3 * MUL]
            D_s = Wout_s[:, 3 * MUL : 4 * MUL]

            # ---- phase A: node-major table -------------------------------
            # Tn[p, blk, :] = [s | vx | vy | vz] row of node (blk*128 + p)
            nblk = (n_nodes + 127) // 128
            Tn = tabs.tile([128, nblk, 4 * MUL], bf16)

            wcfg = [WupS_s, WupV_s, WupV_s, WupV_s]
            evac_flip = 0
            for c in range(nblk):
                n0 = c * 128
                ch = min(128, n_nodes - n0)
                pp = psum.tile([128, 4 * MUL], f32, tag="pso", bufs=3)
                xa = work.tile([128, 4, 128], bf16, tag="nfc", bufs=2)
                # one DMA: xa[p, k, :] = nfT[128k + p, n0:n0+ch]
                nc.sync.dma_start(
                    out=xa[:, :, :ch],
                    in_=nfT.rearrange("(k p) n -> p k n", k=4)[:, :, n0 : n0 + ch],
                )
                for k in range(4):
                    nc.tensor.matmul(
                        pp[:ch, 128 * k : 128 * (k + 1)],
                        lhsT=xa[:, k, :ch],
                        rhs=wcfg[k][:],
                        start=True,
                        stop=True,
                    )
                dst = Tn[:ch, c, :]
                if evac_flip % 2 == 0:
                    nc.scalar.activation(dst, pp[:ch, :], AF.Copy)
                else:
                    nc.vector.tensor_copy(out=dst, in_=pp[:ch, :])
                evac_flip += 1

            # ---- phase B: edge tiles -------------------------------------
            for t in range(nt):
                e0 = t * f
                c0 = t * (f // 16)

                # gather sender rows via SWDGE dma_gather with transpose:
                # out G1[p, c, i] = row(idx_i)[c*128 + p]  -> planar blocks
                G1 = work.tile([128, 4, f], bf16, tag="G1", bufs=3)
                nc.gpsimd.dma_gather(
                    G1[:],
                    Tn[:],
                    idx_s[:, c0 : c0 + f // 16],
                    num_idxs=f,
                    num_idxs_reg=f,
                    elem_size=4 * MUL,
                    transpose=True,
                    sbuf_tokens_per_rank=128,
                    sbuf_free_dim_per_rank=4 * MUL * 2,
                    sbuf_free_dim_pad_per_rank=0,
                    sbuf_byte_offset=0,
                )
                s1 = G1[:, 0, :]
                vx = G1[:, 1, :]
                vy = G1[:, 2, :]
                vz = G1[:, 3, :]

                et = work.tile([EDGE_FEAT_DIM, f], bf16, tag="et", bufs=3)
                nc.sync.dma_start(out=et[:], in_=efT_d[:, e0 : e0 + f])
                yt = work.tile([1, 4 * f], bf16, tag="yt", bufs=3)
                nc.sync.dma_start(out=yt[:], in_=yT_d[:, 4 * e0 : 4 * (e0 + f)])

                # y broadcasts: rank-1 matmuls; y0 read from psum by h3y0
                py0 = psum.tile([64, f], f32, tag="psy", bufs=2)
                nc.tensor.matmul(
                    py0[:], lhsT=ones_s[:, 0:64], rhs=yt[:, 0:f],
                    start=True, stop=True,
                )
                y1b = []
                for m in range(3):
                    pym = psum.tile([128, f], f32, tag="psy", bufs=2)
                    nc.tensor.matmul(
                        pym[:], lhsT=ones_s[:],
                        rhs=yt[:, (1 + m) * f : (2 + m) * f],
                        start=True, stop=True,
                    )
                    ym = work.tile([128, f], bf16, tag=f"y1b{m}", bufs=3)
                    nc.scalar.activation(ym[:], pym[:], AF.Copy)
                    y1b.append(ym)

                # dot = sum_m v1m * y1m   (feature-major)
                dx = work.tile([128, f], bf16, tag="dx", bufs=3)
                nc.vector.tensor_mul(out=dx[:], in0=vx, in1=y1b[0][:])
                dy = work.tile([128, f], bf16, tag="dy", bufs=3)
                nc.vector.tensor_mul(out=dy[:], in0=vy, in1=y1b[1][:])
                dz = work.tile([128, f], bf16, tag="dz", bufs=3)
                nc.vector.tensor_mul(out=dz[:], in0=vz, in1=y1b[2][:])
                dsum = work.tile([128, f], bf16, tag="dsum", bufs=3)
                nc.vector.tensor_add(out=dsum[:], in0=dx[:], in1=dy[:])
                dot = work.tile([128, f], bf16, tag="dot", bufs=3)
                nc.vector.tensor_add(out=dot[:], in0=dsum[:], in1=dz[:])

                # radial MLP (feature-major; silu const folded into W2..W4)
                ph1 = psum.tile([HIDDEN, f], f32, tag="psh", bufs=1)
                nc.tensor.matmul(
                    ph1[:], lhsT=W1_s[:], rhs=et[:],
                    start=True, stop=True,
                )
                h1 = work.tile([HIDDEN, f], bf16, tag="h1", bufs=3)
                nc.scalar.activation(h1[:], ph1[:], AF.Silu)
                ph2 = psum.tile([HIDDEN, f], f32, tag="psh", bufs=1)
                nc.tensor.matmul(
                    ph2[:], lhsT=W2_s[:], rhs=h1[:],
                    start=True, stop=True,
                )
                h2 = work.tile([HIDDEN, f], bf16, tag="h2", bufs=3)
                nc.scalar.activation(h2[:], ph2[:], AF.Silu)
                ph3 = psum.tile([HIDDEN, f], f32, tag="psh", bufs=1)
                nc.tensor.matmul(
                    ph3[:], lhsT=W3_s[:], rhs=h2[:],
                    start=True, stop=True,
                )
                h3 = work.tile([HIDDEN, f], bf16, tag="h3", bufs=3)
                nc.scalar.activation(h3[:], ph3[:], AF.Silu)

                # h3 * y0 (folds y0 into the w_a and w_d tensor-product paths)
                h3y0 = work.tile([HIDDEN, f], bf16, tag="h3y0", bufs=3)
                nc.vector.tensor_mul(out=h3y0[:], in0=h3[:], in1=py0[:])

                # tpw chunks: a,d use h3*y0 (pre-scaled); b,c use h3
                pwa = psum.tile([128, f], f32, tag="psw", bufs=2)
                nc.tensor.matmul(
                    pwa[:], lhsT=W4_s[:, 0:128],
                    rhs=h3y0[:], start=True, stop=True,
                )
                pprime = work.tile([128, f], bf16, tag="pp", bufs=3)
                nc.vector.tensor_mul(out=pprime[:], in0=pwa[:], in1=s1)

                pwd = psum.tile([128, f], f32, tag="psw", bufs=2)
                nc.tensor.matmul(
                    pwd[:], lhsT=W4_s[:, 384:512],
                    rhs=h3y0[:], start=True, stop=True,
                )
                wdy0 = work.tile([128, f], bf16, tag="wdy0", bufs=3)
                nc.scalar.activation(wdy0[:], pwd[:], AF.Copy)

                pwb = psum.tile([128, f], f32, tag="psw", bufs=2)
                nc.tensor.matmul(
                    pwb[:], lhsT=W4_s[:, 128:256],
                    rhs=h3[:], start=True, stop=True,
                )
                rbar = work.tile([128, f], bf16, tag="rbar", bufs=3)
                nc.vector.tensor_mul(out=rbar[:], in0=pwb[:], in1=dot[:])

                pwc = psum.tile([128, f], f32, tag="psw", bufs=2)
                nc.tensor.matmul(
                    pwc[:], lhsT=W4_s[:, 256:384],
                    rhs=h3[:], start=True, stop=True,
                )
                zt = work.tile([128, f], bf16, tag="zt", bufs=3)
                nc.vector.tensor_mul(out=zt[:], in0=pwc[:], in1=s1)

                q_m = []
                for m in range(3):
                    qm = work.tile([128, f], bf16, tag=f"q{m}", bufs=3)
                    nc.vector.tensor_mul(out=qm[:], in0=zt[:], in1=y1b[m][:])
                    q_m.append(qm)
                t_m = []
                for m, vcomp in enumerate((vx, vy, vz)):
                    tm = work.tile([128, f], bf16, tag=f"t{m}", bufs=3)
                    nc.vector.tensor_mul(out=tm[:], in0=wdy0[:], in1=vcomp)
                    t_m.append(tm)

                # final linear (feature-major out), psum-accumulated pairs
                psS = psum.tile([128, f], f32, tag="pso", bufs=3)
                nc.tensor.matmul(psS[:], lhsT=A_s, rhs=pprime[:], start=True, stop=False)
                nc.tensor.matmul(psS[:], lhsT=B_s, rhs=rbar[:], start=False, stop=True)
                outS = work.tile([128, f], f32, tag="oS", bufs=3)
                nc.scalar.activation(outS[:], psS[:], AF.Copy)
                nc.sync.dma_start(out=outT_d[0:128, e0 : e0 + f], in_=outS[:])

                for m in range(3):
                    psV = psum.tile([128, f], f32, tag="pso", bufs=3)
                    nc.tensor.matmul(psV[:], lhsT=C_s, rhs=q_m[m][:], start=True, stop=False)
                    nc.tensor.matmul(psV[:], lhsT=D_s, rhs=t_m[m][:], start=False, stop=True)
                    outV = work.tile([128, f], f32, tag=f"oV{m}", bufs=3)
                    if m == 1:
                        nc.vector.tensor_copy(out=outV[:], in_=psV[:])
                    else:
                        nc.scalar.activation(outV[:], psV[:], AF.Copy)
                    nc.sync.dma_start(
                        out=outT_d[128 * (m + 1) : 128 * (m + 2), e0 : e0 + f],
                        in_=outV[:],
                    )

    nc.compile()
    return nc


def prep_host_inputs(node_feats, edge_index, edge_attrs, edge_feats,
                     W_up_s, W_up_v, W1, W2, W3, W4, W_out_s, W_out_v,
                     n_nodes=N_NODES, f=F, nt=NT, n_cores=N_CORES):
    """Fold constants, build device layouts, shard edges. Returns in_maps."""
    import ml_dtypes

    cst = _silu_cst()
    node_feats = np.asarray(node_feats, dtype=np.float32)
    edge_attrs = np.asarray(edge_attrs, dtype=np.float32)
    edge_feats = np.asarray(edge_feats, dtype=np.float32)
    sender = np.asarray(edge_index)[0].astype(np.int64)

    esp = nt * f
    n_edges = sender.shape[0]
    es = n_edges // n_cores

    # weights with all norm constants folded
    W1h = (np.asarray(W1, np.float32) / np.sqrt(np.float32(EDGE_FEAT_DIM)))
    W2h = (np.asarray(W2, np.float32) / np.sqrt(np.float32(HIDDEN))) * cst
    W3h = (np.asarray(W3, np.float32) / np.sqrt(np.float32(HIDDEN))) * cst
    W4h = (np.asarray(W4, np.float32) / np.sqrt(np.float32(HIDDEN))) * cst
    inv_sqrt_mul = np.float32(1.0 / np.sqrt(MUL))
    WupSh = np.asarray(W_up_s, np.float32) * inv_sqrt_mul
    WupVh = np.asarray(W_up_v, np.float32) * inv_sqrt_mul
    inv2 = np.float32(1.0 / np.sqrt(2 * MUL))
    A = np.asarray(W_out_s, np.float32)[:MUL] * inv2
    B = np.asarray(W_out_s, np.float32)[MUL:] * (inv2 / np.sqrt(np.float32(3.0)))
    C = np.asarray(W_out_v, np.float32)[:MUL] * inv2
    D = np.asarray(W_out_v, np.float32)[MUL:] * inv2
    Wout = np.concatenate([A, B, C, D], axis=1).astype(ml_dtypes.bfloat16)

    # node features, feature-major planes: s, vx, vy, vz
    nfT = np.empty((4, MUL, n_nodes), np.float32)
    nfT[0] = node_feats[:, :MUL].T
    for m in range(3):
        nfT[1 + m] = node_feats[:, MUL + m :: 3].T
    nfT = np.ascontiguousarray(nfT.reshape(4 * MUL, n_nodes))

    ones = np.ones((1, MUL), np.float32)

    bf = ml_dtypes.bfloat16
    shared = {
        "nfT": np.ascontiguousarray(nfT.astype(bf)),
        "W1": np.ascontiguousarray(W1h.astype(bf)),
        "W2": np.ascontiguousarray(W2h.astype(bf)),
        "W3": np.ascontiguousarray(W3h.astype(bf)),
        "W4": np.ascontiguousarray(W4h.astype(bf)),
        "WupS": np.ascontiguousarray(WupSh.astype(bf)),
        "WupV": np.ascontiguousarray(WupVh.astype(bf)),
        "Wout": np.ascontiguousarray(Wout),
        "ones": ones.astype(bf),
    }

    in_maps = []
    for c in range(n_cores):
        lo, hi = c * es, (c + 1) * es
        snd = np.zeros(esp, np.int16)
        snd[: es] = sender[lo:hi].astype(np.int16)
        # ap_gather layout: idx[16g+p, t*(f//16)+s] = snd[t*f + s*16 + p]
        sp = snd.reshape(nt, f // 16, 16)           # [t, s, p]
        grid16 = sp.transpose(2, 0, 1).reshape(16, nt * (f // 16))
        idx_l = np.ascontiguousarray(np.tile(grid16, (8, 1)))

        efT = np.zeros((EDGE_FEAT_DIM, esp), np.float32)
        efT[:, :es] = edge_feats[lo:hi].T
        efT = efT.astype(ml_dtypes.bfloat16)
        yT = np.zeros((4, esp), np.float32)
        yT[:, :es] = edge_attrs[lo:hi].T
        # per-tile flat layout: [1, t*4f + r*f + e] so each rank-1 broadcast
        # rhs slice starts at partition 0
        y_flat = np.ascontiguousarray(
            yT.reshape(4, nt, f).transpose(1, 0, 2).reshape(1, 4 * esp)
        ).astype(ml_dtypes.bfloat16)

        in_maps.append(dict(shared, idx=idx_l, efT=efT, yT=y_flat))
    return in_maps


_PROG_CACHE = {}


def _run_pjrt(nc, in_maps, n_cores=N_CORES, time_reps=0, profile_dir=None):
    """Execute the SPMD program via PJRT. Returns (results, wall_times)."""
    import time as _time

    import jax
    from jax.sharding import Mesh, NamedSharding, PartitionSpec

    try:
        from jax.experimental.shard_map import shard_map
    except ImportError:  # newer jax
        from jax.sharding import shard_map
    from concourse import bass2jax, mybir

    bass2jax.install_neuronx_cc_hook()

    save_neff = os.environ.get("KERNEL_SAVE_NEFF")
    if save_neff:
        _orig_rename = bass2jax.rename_neff_tensors_and_patch_header.__wrapped__ if hasattr(
            bass2jax.rename_neff_tensors_and_patch_header, "__wrapped__"
        ) else bass2jax.rename_neff_tensors_and_patch_header

        def _rename_and_save(neff_file, renames):
            data = _orig_rename(neff_file, renames)
            with open(save_neff, "wb") as fh:
                fh.write(data)
            return data

        bass2jax.rename_neff_tensors_and_patch_header = _rename_and_save

    partition_name = (
        nc.partition_id_tensor.name if nc.partition_id_tensor is not None else None
    )
    in_names, out_names, out_avals, zero_outs = [], [], [], []
    for alloc in nc.m.functions[0].allocations:
        if not isinstance(alloc, mybir.MemoryLocationSet):
            continue
        name = alloc.memorylocations[0].name
        if alloc.kind == "ExternalInput":
            if name != partition_name:
                in_names.append(name)
        elif alloc.kind == "ExternalOutput":
            shape = tuple(alloc.tensor_shape)
            dtype = mybir.dt.np(alloc.dtype)
            out_names.append(name)
            out_avals.append(jax.core.ShapedArray(shape, dtype))
            zero_outs.append(np.zeros(shape, dtype))
    n_params = len(in_names)
    in_names_all = in_names + out_names
    if partition_name is not None:
        in_names_all = in_names_all + [partition_name]

    def _body(*args):
        operands = list(args)
        if partition_name is not None:
            operands.append(bass2jax.partition_id_tensor())
        outs = bass2jax._bass_exec_p.bind(
            *operands,
            out_avals=tuple(out_avals),
            in_names=tuple(in_names_all),
            out_names=tuple(out_names),
            lowering_input_output_aliases=(),
            sim_require_finite=True,
            sim_require_nnan=True,
            nc=nc,
        )
        return tuple(outs)

    devices = jax.devices()[:n_cores]
    mesh = Mesh(np.asarray(devices), ("core",))
    nouts = len(out_names)
    donate = tuple(range(n_params, n_params + nouts))
    sharded = jax.jit(
        shard_map(
            _body,
            mesh=mesh,
            in_specs=(PartitionSpec("core"),) * (n_params + nouts),
            out_specs=(PartitionSpec("core"),) * nouts,
            check_rep=False,
        ),
        donate_argnums=donate,
        keep_unused=True,
    )

    spec = NamedSharding(mesh, PartitionSpec("core"))
    dev_in = [
        jax.device_put(
            np.concatenate([np.asarray(in_maps[c][nm]) for c in range(n_cores)], axis=0),
            spec,
        )
        for nm in in_names
    ]

    def make_zeros():
        return [
            jax.device_put(np.zeros((n_cores * z.shape[0], *z.shape[1:]), z.dtype), spec)
            for z in zero_outs
        ]

    out_arrs = jax.block_until_ready(sharded(*dev_in, *make_zeros()))

    times = []
    prof_ctx = None
    if profile_dir:
        prof_ctx = _ntff_profiler()
    for r in range(max(time_reps, 0)):
        zs = make_zeros()
        jax.block_until_ready(zs)
        do_prof = prof_ctx is not None and r == time_reps - 1
        if do_prof:
            prof_ctx.start()
        t0 = _time.perf_counter()
        out_arrs = jax.block_until_ready(sharded(*dev_in, *zs))
        times.append(_time.perf_counter() - t0)
        if do_prof:
            prof_ctx.stop(profile_dir)

    results = [
        {
            nm: np.asarray(out_arrs[i]).reshape(n_cores, *out_avals[i].shape)[c]
            for i, nm in enumerate(out_names)
        }
        for c in range(n_cores)
    ]
    return results, times


class _ntff_profiler:
    def __init__(self, so_path="/opt/axon/libaxon_pjrt.so"):
        import ctypes

        self.lib = ctypes.CDLL(so_path)
        self.ctypes = ctypes
        self.lib.axon_start_nrt_profile.argtypes = [
            ctypes.POINTER(ctypes.c_int64),
            ctypes.c_size_t,
        ]
        self.lib.axon_start_nrt_profile.restype = ctypes.c_int64
        self.lib.axon_stop_nrt_profile.argtypes = [ctypes.c_char_p]
        self.lib.axon_stop_nrt_profile.restype = ctypes.c_int64

    def start(self):
        rc = self.lib.axon_start_nrt_profile(None, 0)
        if rc != 0:
            print(f"ntff profile start failed rc={rc}")

    def stop(self, outdir):
        os.makedirs(outdir, exist_ok=True)
        n = self.lib.axon_stop_nrt_profile(str(outdir).encode())
        print(f"ntff profile: {n} file(s) -> {outdir}")


def kernel(node_feats, edge_index, edge_attrs, edge_feats,
           W_up_s, W_up_v, W1, W2, W3, W4, W_out_s, W_out_v):
    in_maps = prep_host_inputs(
        node_feats, edge_index, edge_attrs, edge_feats,
        W_up_s, W_up_v, W1, W2, W3, W4, W_out_s, W_out_v,
    )

    key = (N_NODES, F, NT)
    if key not in _PROG_CACHE:
        _PROG_CACHE[key] = build_program(N_NODES, F, NT)
    nc = _PROG_CACHE[key]

    time_reps = int(os.environ.get("KERNEL_TIME_REPS", "0"))
    profile_dir = os.environ.get("KERNEL_PROFILE_DIR") or None
    results, times = _run_pjrt(
        nc, in_maps, N_CORES, time_reps=time_reps, profile_dir=profile_dir
    )
    if times:
        best = min(times)
        kernel.last_exec_time_ns = int(best * 1e9)
        kernel.last_times = times
        print(f"wall times (s): {[f'{x:.6f}' for x in times]}")

    out = np.empty((N_EDGES, 4 * MUL), np.float32)
    for c in range(N_CORES):
        ot = np.asarray(results[c]["outT"], np.float32)[:, :ES]   # [512, ES]
        lo = c * ES
        out[lo : lo + ES, :MUL] = ot[:MUL].T
        out[lo : lo + ES, MUL:] = (
            ot[MUL:].reshape(3, MUL, ES).transpose(2, 1, 0).reshape(ES, 3 * MUL)
        )
    return out



# revision 2
# speedup vs baseline: 2.5853x; 2.5853x over previous
"""Trainium2 Bass kernel: e3nn edge message block (gnn_message_passing).

Strategy (edge-parallel across 8 cores, v2):
  - Host: sort edges by sender; each core gets 25000 contiguous sorted
    edges whose senders span a <=2048-node window -> per-core node table
    is 16 blocks instead of 79. Norm constants folded into weights.
  - Device phase A: local node table Tn[p, blk, :] = [s|vx|vy|vz] row of
    window-node (blk*128+p), built with bf16 matmuls.
  - Device phase B, per 512-edge tile (feature-major [128, 512]):
      * gpsimd dma_gather pulls per-edge sender rows from SBUF table
      * y0/y1 broadcasts via ONE partition-broadcast DMA from DRAM
      * radial MLP on PE + ACT silu; uvu tensor product on DVE with
        [128, 3*512] fused ops (stride-0 broadcasts)
      * final linear: 8 accumulating matmuls into two 2-bank PSUM tiles
      * bf16 output, 2 DMAs per tile; host inverse-permutes + upcasts
"""

import os
import sys

sys.path.insert(0, "/opt/trn_rl_repo")

import numpy as np

MUL = 128
N_NODES = 10000
N_EDGES = 200000
N_CORES = 8
ES = N_EDGES // N_CORES          # 25000 edges per core
F = 512                          # edges per tile (free dim)
NT = (ES + F - 1) // F           # 49 tiles
ESP = NT * F                     # 25088 padded edges per core
EDGE_FEAT_DIM = 8
HIDDEN = 64
NWIN = 2048                      # per-core node window (16 blocks of 128)
NBLK = NWIN // 128


def _silu_cst():
    z = np.linspace(-12.0, 12.0, 200001)
    pdf = np.exp(-0.5 * z * z) / np.sqrt(2.0 * np.pi)
    silu = z / (1.0 + np.exp(-z))
    trapz = getattr(np, "trapezoid", None) or getattr(np, "trapz")
    return np.float32(1.0 / np.sqrt(trapz(silu * silu * pdf, z)))


def build_program(f=F, nt=NT):
    """Build the SPMD single-core Bass program (same program on all cores)."""
    import concourse.bass as bass
    import concourse.bacc as bacc
    import concourse.tile as tile
    from concourse import mybir

    f32 = mybir.dt.float32
    bf16 = mybir.dt.bfloat16
    i16 = mybir.dt.int16
    AF = mybir.ActivationFunctionType

    esp = nt * f
    nc = bacc.Bacc(None, target_bir_lowering=False, debug=False)

    # ---- DRAM parameters --------------------------------------------------
    nfT = nc.declare_dram_parameter("nfT", [4 * MUL, NWIN], bf16, isOutput=False)
    idx_d = nc.declare_dram_parameter("idx", [128, nt * (f // 16)], i16, isOutput=False)
    efT_d = nc.declare_dram_parameter("efT", [EDGE_FEAT_DIM, esp], bf16, isOutput=False)
    yT_d = nc.declare_dram_parameter("yT", [1, 4 * esp], bf16, isOutput=False)
    W1_d = nc.declare_dram_parameter("W1", [EDGE_FEAT_DIM, HIDDEN], bf16, isOutput=False)
    W2_d = nc.declare_dram_parameter("W2", [HIDDEN, HIDDEN], bf16, isOutput=False)
    W3_d = nc.declare_dram_parameter("W3", [HIDDEN, HIDDEN], bf16, isOutput=False)
    W4_d = nc.declare_dram_parameter("W4", [HIDDEN, 4 * MUL], bf16, isOutput=False)
    WupS_d = nc.declare_dram_parameter("WupS", [MUL, MUL], bf16, isOutput=False)
    WupV_d = nc.declare_dram_parameter("WupV", [MUL, MUL], bf16, isOutput=False)
    Wout_d = nc.declare_dram_parameter("Wout", [MUL, 4 * MUL], bf16, isOutput=False)
    outT_d = nc.declare_dram_parameter("outT", [4 * MUL, esp], bf16, isOutput=True)

    with tile.TileContext(nc) as tc:
        with (
            tc.tile_pool(name="const", bufs=1) as const,
            tc.tile_pool(name="tables", bufs=1) as tabs,
            tc.tile_pool(name="work", bufs=2) as work,
            tc.tile_pool(name="psum", bufs=2, space="PSUM") as psum,
        ):
            # ---- constants into SBUF -------------------------------------
            def cload(dram, shape, dtype, name):
                t = const.tile(shape, dtype, name=name, tag=name)
                nc.sync.dma_start(out=t[:], in_=dram[:])
                return t

            W1_s = cload(W1_d, [EDGE_FEAT_DIM, HIDDEN], bf16, "cW1")
            W2_s = cload(W2_d, [HIDDEN, HIDDEN], bf16, "cW2")
            W3_s = cload(W3_d, [HIDDEN, HIDDEN], bf16, "cW3")
            W4_s = cload(W4_d, [HIDDEN, 4 * MUL], bf16, "cW4")
            WupS_s = cload(WupS_d, [MUL, MUL], bf16, "cWupS")
            WupV_s = cload(WupV_d, [MUL, MUL], bf16, "cWupV")
            Wout_s = cload(Wout_d, [MUL, 4 * MUL], bf16, "cWout")  # A|B|C|D
            idx_s = const.tile([128, nt * (f // 16)], i16, name="cidx", tag="cidx")
            nc.sync.dma_start(out=idx_s[:], in_=idx_d[:])

            A_s = Wout_s[:, 0:MUL]
            B_s = Wout_s[:, MUL : 2 * MUL]
            C_s = Wout_s[:, 2 * MUL : 3 * MUL]
            D_s = Wout_s[:, 3 * MUL : 4 * MUL]

            # ---- phase A: node-major table (local window) ----------------
            # Tn[p, blk, :] = [s | vx | vy | vz] row of node (blk*128 + p)
            Tn = tabs.tile([128, NBLK, 4 * MUL], bf16)

            wcfg = [WupS_s, WupV_s, WupV_s, WupV_s]
            for c in range(NBLK):
                n0 = c * 128
                pp = psum.tile([128, 4 * MUL], f32, tag="pw", bufs=4)
                xa = work.tile([128, 4, 128], bf16, tag="nfc", bufs=2)
                # one DMA: xa[p, k, :] = nfT[128k + p, n0:n0+128]
                nc.sync.dma_start(
                    out=xa[:],
                    in_=nfT.rearrange("(k p) n -> p k n", k=4)[:, :, n0 : n0 + 128],
                )
                for k in range(4):
                    nc.tensor.matmul(
                        pp[:, 128 * k : 128 * (k + 1)],
                        lhsT=xa[:, k, :],
                        rhs=wcfg[k][:],
                        start=True,
                        stop=True,
                    )
                nc.scalar.activation(Tn[:, c, :], pp[:], AF.Copy)

            # ---- phase B: edge tiles -------------------------------------
            for t in range(nt):
                e0 = t * f
                c0 = t * (f // 16)

                # gather sender rows: G1[p, k, i] = row(idx_i)[k*128 + p]
                G1 = work.tile([128, 4, f], bf16, tag="G1", bufs=3)
                nc.gpsimd.dma_gather(
                    G1[:],
                    Tn[:],
                    idx_s[:, c0 : c0 + f // 16],
                    num_idxs=f,
                    num_idxs_reg=f,
                    elem_size=4 * MUL,
                    transpose=True,
                    sbuf_tokens_per_rank=128,
                    sbuf_free_dim_per_rank=4 * MUL * 2,
                    sbuf_free_dim_pad_per_rank=0,
                    sbuf_byte_offset=0,
                )
                s1 = G1[:, 0, :]
                v3 = G1[:, 1:4, :]

                et = work.tile([EDGE_FEAT_DIM, f], bf16, tag="et", bufs=3)
                nc.sync.dma_start(out=et[:], in_=efT_d[:, e0 : e0 + f])

                # all 4 per-edge scalars broadcast in one DMA:
                # yb[p, :] = [y0 | y1x | y1y | y1z] (f each)
                yb = work.tile([128, 4 * f], bf16, tag="yb", bufs=3)
                nc.sync.dma_start(
                    out=yb[:],
                    in_=yT_d[0:1, 4 * e0 : 4 * (e0 + f)].partition_broadcast(128),
                )
                y1b3 = yb[:, f : 4 * f].rearrange("p (m f) -> p m f", m=3)

                # radial MLP (feature-major; silu const folded into W2..W4)
                ph1 = psum.tile([128, f], f32, tag="pw", bufs=4)
                nc.tensor.matmul(ph1[0:HIDDEN, :], lhsT=W1_s[:], rhs=et[:],
                                 start=True, stop=True)
                h1 = work.tile([HIDDEN, f], bf16, tag="h1", bufs=2)
                nc.scalar.activation(h1[:], ph1[0:HIDDEN, :], AF.Silu)
                ph2 = psum.tile([128, f], f32, tag="pw", bufs=4)
                nc.tensor.matmul(ph2[0:HIDDEN, :], lhsT=W2_s[:], rhs=h1[:],
                                 start=True, stop=True)
                h2 = work.tile([HIDDEN, f], bf16, tag="h2", bufs=2)
                nc.scalar.activation(h2[:], ph2[0:HIDDEN, :], AF.Silu)
                ph3 = psum.tile([128, f], f32, tag="pw", bufs=4)
                nc.tensor.matmul(ph3[0:HIDDEN, :], lhsT=W3_s[:], rhs=h2[:],
                                 start=True, stop=True)
                h3 = work.tile([HIDDEN, f], bf16, tag="h3", bufs=2)
                nc.scalar.activation(h3[:], ph3[0:HIDDEN, :], AF.Silu)

                # h3 * y0 (folds y0 into the w_a and w_d tensor-product paths)
                h3y0 = work.tile([HIDDEN, f], bf16, tag="h3y0", bufs=2)
                nc.vector.tensor_mul(out=h3y0[:], in0=h3[:], in1=yb[0:HIDDEN, 0:f])

                # tpw chunks: a,d use h3*y0 (pre-scaled); b,c use h3
                pwa = psum.tile([128, f], f32, tag="pw", bufs=4)
                nc.tensor.matmul(pwa[:], lhsT=W4_s[:, 0:128], rhs=h3y0[:],
                                 start=True, stop=True)
                pwd = psum.tile([128, f], f32, tag="pw", bufs=4)
                nc.tensor.matmul(pwd[:], lhsT=W4_s[:, 384:512], rhs=h3y0[:],
                                 start=True, stop=True)
                pwb = psum.tile([128, f], f32, tag="pw", bufs=4)
                nc.tensor.matmul(pwb[:], lhsT=W4_s[:, 128:256], rhs=h3[:],
                                 start=True, stop=True)
                pwc = psum.tile([128, f], f32, tag="pw", bufs=4)
                nc.tensor.matmul(pwc[:], lhsT=W4_s[:, 256:384], rhs=h3[:],
                                 start=True, stop=True)

                wdy0 = work.tile([128, f], bf16, tag="wdy0", bufs=2)
                nc.scalar.activation(wdy0[:], pwd[:], AF.Copy)

                # dot = sum_m v1m * y1m
                D3 = work.tile([128, 3 * f], bf16, tag="D3", bufs=2)
                nc.vector.tensor_mul(
                    out=D3[:].rearrange("p (m f) -> p m f", m=3),
                    in0=v3, in1=y1b3,
                )
                dsum = work.tile([128, f], bf16, tag="dsum", bufs=2)
                nc.vector.tensor_add(out=dsum[:], in0=D3[:, 0:f], in1=D3[:, f : 2 * f])
                dot = work.tile([128, f], bf16, tag="dot", bufs=2)
                nc.vector.tensor_add(out=dot[:], in0=dsum[:], in1=D3[:, 2 * f : 3 * f])

                pprime = work.tile([128, f], bf16, tag="pp", bufs=2)
                nc.vector.tensor_mul(out=pprime[:], in0=pwa[:], in1=s1)
                rbar = work.tile([128, f], bf16, tag="rbar", bufs=2)
                nc.vector.tensor_mul(out=rbar[:], in0=pwb[:], in1=dot[:])
                zt = work.tile([128, f], bf16, tag="zt", bufs=2)
                nc.vector.tensor_mul(out=zt[:], in0=pwc[:], in1=s1)

                # q3[p, m, e] = zt * y1m ;  t3[p, m, e] = wdy0 * v1m
                q3 = work.tile([128, 3 * f], bf16, tag="q3", bufs=2)
                nc.vector.tensor_mul(
                    out=q3[:].rearrange("p (m f) -> p m f", m=3),
                    in0=zt[:].unsqueeze(1).broadcast_to([128, 3, f]),
                    in1=y1b3,
                )
                t3 = work.tile([128, 3 * f], bf16, tag="t3", bufs=2)
                nc.vector.tensor_mul(
                    out=t3[:].rearrange("p (m f) -> p m f", m=3),
                    in0=wdy0[:].unsqueeze(1).broadcast_to([128, 3, f]),
                    in1=v3,
                )

                # final linear: [s|vx] into outA, [vy|vz] into outB
                outA = psum.tile([128, 2 * f], f32, tag="oA", bufs=1)
                nc.tensor.matmul(outA[:, 0:f], lhsT=A_s, rhs=pprime[:],
                                 start=True, stop=False)
                nc.tensor.matmul(outA[:, 0:f], lhsT=B_s, rhs=rbar[:],
                                 start=False, stop=True)
                nc.tensor.matmul(outA[:, f : 2 * f], lhsT=C_s, rhs=q3[:, 0:f],
                                 start=True, stop=False)
                nc.tensor.matmul(outA[:, f : 2 * f], lhsT=D_s, rhs=t3[:, 0:f],
                                 start=False, stop=True)
                outB = psum.tile([128, 2 * f], f32, tag="oB", bufs=1)
                nc.tensor.matmul(outB[:, 0:f], lhsT=C_s, rhs=q3[:, f : 2 * f],
                                 start=True, stop=False)
                nc.tensor.matmul(outB[:, 0:f], lhsT=D_s, rhs=t3[:, f : 2 * f],
                                 start=False, stop=True)
                nc.tensor.matmul(outB[:, f : 2 * f], lhsT=C_s, rhs=q3[:, 2 * f : 3 * f],
                                 start=True, stop=False)
                nc.tensor.matmul(outB[:, f : 2 * f], lhsT=D_s, rhs=t3[:, 2 * f : 3 * f],
                                 start=False, stop=True)

                sbA = work.tile([128, 2 * f], bf16, tag="sbA", bufs=3)
                nc.scalar.activation(sbA[:], outA[:], AF.Copy)
                sbB = work.tile([128, 2 * f], bf16, tag="sbB", bufs=3)
                nc.vector.tensor_copy(out=sbB[:], in_=outB[:])

                od = outT_d.rearrange("(c p) e -> p c e", c=4)
                nc.sync.dma_start(
                    out=od[:, 0:2, e0 : e0 + f],
                    in_=sbA[:].rearrange("p (c f) -> p c f", c=2),
                )
                nc.sync.dma_start(
                    out=od[:, 2:4, e0 : e0 + f],
                    in_=sbB[:].rearrange("p (c f) -> p c f", c=2),
                )

    nc.compile()
    return nc


def prep_host_inputs(node_feats, edge_index, edge_attrs, edge_feats,
                     W_up_s, W_up_v, W1, W2, W3, W4, W_out_s, W_out_v,
                     f=F, nt=NT, n_cores=N_CORES):
    """Sort edges, fold constants, build device layouts. Returns (in_maps, perm)."""
    import ml_dtypes

    cst = _silu_cst()
    node_feats = np.asarray(node_feats, dtype=np.float32)
    edge_attrs = np.asarray(edge_attrs, dtype=np.float32)
    edge_feats = np.asarray(edge_feats, dtype=np.float32)
    sender = np.asarray(edge_index)[0].astype(np.int64)

    # sort edges by sender: each core gets a contiguous node window
    perm = np.argsort(sender, kind="stable")
    sender_s = sender[perm]
    edge_attrs = edge_attrs[perm]
    edge_feats = edge_feats[perm]

    esp = nt * f
    n_edges = sender.shape[0]
    es = n_edges // n_cores

    # weights with all norm constants folded
    W1h = (np.asarray(W1, np.float32) / np.sqrt(np.float32(EDGE_FEAT_DIM)))
    W2h = (np.asarray(W2, np.float32) / np.sqrt(np.float32(HIDDEN))) * cst
    W3h = (np.asarray(W3, np.float32) / np.sqrt(np.float32(HIDDEN))) * cst
    W4h = (np.asarray(W4, np.float32) / np.sqrt(np.float32(HIDDEN))) * cst
    inv_sqrt_mul = np.float32(1.0 / np.sqrt(MUL))
    WupSh = np.asarray(W_up_s, np.float32) * inv_sqrt_mul
    WupVh = np.asarray(W_up_v, np.float32) * inv_sqrt_mul
    inv2 = np.float32(1.0 / np.sqrt(2 * MUL))
    A = np.asarray(W_out_s, np.float32)[:MUL] * inv2
    B = np.asarray(W_out_s, np.float32)[MUL:] * (inv2 / np.sqrt(np.float32(3.0)))
    C = np.asarray(W_out_v, np.float32)[:MUL] * inv2
    D = np.asarray(W_out_v, np.float32)[MUL:] * inv2
    Wout = np.concatenate([A, B, C, D], axis=1).astype(ml_dtypes.bfloat16)

    # node features, feature-major planes: s, vx, vy, vz
    nfT = np.empty((4, MUL, N_NODES), np.float32)
    nfT[0] = node_feats[:, :MUL].T
    for m in range(3):
        nfT[1 + m] = node_feats[:, MUL + m :: 3].T
    nfT = np.ascontiguousarray(nfT.reshape(4 * MUL, N_NODES))

    bf = ml_dtypes.bfloat16
    shared = {
        "W1": np.ascontiguousarray(W1h.astype(bf)),
        "W2": np.ascontiguousarray(W2h.astype(bf)),
        "W3": np.ascontiguousarray(W3h.astype(bf)),
        "W4": np.ascontiguousarray(W4h.astype(bf)),
        "WupS": np.ascontiguousarray(WupSh.astype(bf)),
        "WupV": np.ascontiguousarray(WupVh.astype(bf)),
        "Wout": np.ascontiguousarray(Wout),
    }

    in_maps = []
    for c in range(n_cores):
        lo, hi = c * es, (c + 1) * es
        snd = sender_s[lo:hi]
        base = int(snd[0])
        width = int(snd[-1]) - base + 1
        assert width <= NWIN, f"core {c}: node window {width} > {NWIN}"
        base = min(base, N_NODES - NWIN)

        nf_slice = np.zeros((4 * MUL, NWIN), np.float32)
        nf_slice[:, : min(NWIN, N_NODES - base)] = nfT[:, base : base + NWIN]

        loc = np.zeros(esp, np.int16)
        loc[:es] = (snd - base).astype(np.int16)
        # gather layout: idx[16g+p, t*(f//16)+s] = loc[t*f + s*16 + p]
        sp = loc.reshape(nt, f // 16, 16)           # [t, s, p]
        grid16 = sp.transpose(2, 0, 1).reshape(16, nt * (f // 16))
        idx_l = np.ascontiguousarray(np.tile(grid16, (8, 1)))

        efT = np.zeros((EDGE_FEAT_DIM, esp), np.float32)
        efT[:, :es] = edge_feats[lo:hi].T
        efT = efT.astype(bf)
        yT = np.zeros((4, esp), np.float32)
        yT[:, :es] = edge_attrs[lo:hi].T
        # per-tile flat layout: [1, t*4f + r*f + e]
        y_flat = np.ascontiguousarray(
            yT.reshape(4, nt, f).transpose(1, 0, 2).reshape(1, 4 * esp)
        ).astype(bf)

        in_maps.append(dict(shared, nfT=nf_slice.astype(bf), idx=idx_l,
                            efT=efT, yT=y_flat))
    return in_maps, perm


_PROG_CACHE = {}


def _run_pjrt(nc, in_maps, n_cores=N_CORES, time_reps=0, profile_dir=None):
    """Execute the SPMD program via PJRT. Returns (results, wall_times)."""
    import time as _time

    import jax
    from jax.sharding import Mesh, NamedSharding, PartitionSpec

    try:
        from jax.experimental.shard_map import shard_map
    except ImportError:  # newer jax
        from jax.sharding import shard_map
    from concourse import bass2jax, mybir

    bass2jax.install_neuronx_cc_hook()

    save_neff = os.environ.get("KERNEL_SAVE_NEFF")
    if save_neff:
        _orig_rename = bass2jax.rename_neff_tensors_and_patch_header.__wrapped__ if hasattr(
            bass2jax.rename_neff_tensors_and_patch_header, "__wrapped__"
        ) else bass2jax.rename_neff_tensors_and_patch_header

        def _rename_and_save(neff_file, renames):
            data = _orig_rename(neff_file, renames)
            with open(save_neff, "wb") as fh:
                fh.write(data)
            return data

        bass2jax.rename_neff_tensors_and_patch_header = _rename_and_save

    partition_name = (
        nc.partition_id_tensor.name if nc.partition_id_tensor is not None else None
    )
    in_names, out_names, out_avals, zero_outs = [], [], [], []
    for alloc in nc.m.functions[0].allocations:
        if not isinstance(alloc, mybir.MemoryLocationSet):
            continue
        name = alloc.memorylocations[0].name
        if alloc.kind == "ExternalInput":
            if name != partition_name:
                in_names.append(name)
        elif alloc.kind == "ExternalOutput":
            shape = tuple(alloc.tensor_shape)
            dtype = mybir.dt.np(alloc.dtype)
            out_names.append(name)
            out_avals.append(jax.core.ShapedArray(shape, dtype))
            zero_outs.append(np.zeros(shape, dtype))
    n_params = len(in_names)
    in_names_all = in_names + out_names
    if partition_name is not None:
        in_names_all = in_names_all + [partition_name]

    def _body(*args):
        operands = list(args)
        if partition_name is not None:
            operands.append(bass2jax.partition_id_tensor())
        outs = bass2jax._bass_exec_p.bind(
            *operands,
            out_avals=tuple(out_avals),
            in_names=tuple(in_names_all),
            out_names=tuple(out_names),
            lowering_input_output_aliases=(),
            sim_require_finite=True,
            sim_require_nnan=True,
            nc=nc,
        )
        return tuple(outs)

    devices = jax.devices()[:n_cores]
    mesh = Mesh(np.asarray(devices), ("core",))
    nouts = len(out_names)
    donate = tuple(range(n_params, n_params + nouts))
    sharded = jax.jit(
        shard_map(
            _body,
            mesh=mesh,
            in_specs=(PartitionSpec("core"),) * (n_params + nouts),
            out_specs=(PartitionSpec("core"),) * nouts,
            check_rep=False,
        ),
        donate_argnums=donate,
        keep_unused=True,
    )

    spec = NamedSharding(mesh, PartitionSpec("core"))
    dev_in = [
        jax.device_put(
            np.concatenate([np.asarray(in_maps[c][nm]) for c in range(n_cores)], axis=0),
            spec,
        )
        for nm in in_names
    ]

    def make_zeros():
        return [
            jax.device_put(np.zeros((n_cores * z.shape[0], *z.shape[1:]), z.dtype), spec)
            for z in zero_outs
        ]

    out_arrs = jax.block_until_ready(sharded(*dev_in, *make_zeros()))

    times = []
    prof_ctx = None
    if profile_dir:
        prof_ctx = _ntff_profiler()
    for r in range(max(time_reps, 0)):
        zs = make_zeros()
        jax.block_until_ready(zs)
        do_prof = prof_ctx is not None and r == time_reps - 1
        if do_prof:
            prof_ctx.start()
        t0 = _time.perf_counter()
        out_arrs = jax.block_until_ready(sharded(*dev_in, *zs))
        times.append(_time.perf_counter() - t0)
        if do_prof:
            prof_ctx.stop(profile_dir)

    results = [
        {
            nm: np.asarray(out_arrs[i]).reshape(n_cores, *out_avals[i].shape)[c]
            for i, nm in enumerate(out_names)
        }
        for c in range(n_cores)
    ]
    return results, times


class _ntff_profiler:
    def __init__(self, so_path="/opt/axon/libaxon_pjrt.so"):
        import ctypes

        self.lib = ctypes.CDLL(so_path)
        self.ctypes = ctypes
        self.lib.axon_start_nrt_profile.argtypes = [
            ctypes.POINTER(ctypes.c_int64),
            ctypes.c_size_t,
        ]
        self.lib.axon_start_nrt_profile.restype = ctypes.c_int64
        self.lib.axon_stop_nrt_profile.argtypes = [ctypes.c_char_p]
        self.lib.axon_stop_nrt_profile.restype = ctypes.c_int64

    def start(self):
        rc = self.lib.axon_start_nrt_profile(None, 0)
        if rc != 0:
            print(f"ntff profile start failed rc={rc}")

    def stop(self, outdir):
        os.makedirs(outdir, exist_ok=True)
        n = self.lib.axon_stop_nrt_profile(str(outdir).encode())
        print(f"ntff profile: {n} file(s) -> {outdir}")


def kernel(node_feats, edge_index, edge_attrs, edge_feats,
           W_up_s, W_up_v, W1, W2, W3, W4, W_out_s, W_out_v):
    in_maps, perm = prep_host_inputs(
        node_feats, edge_index, edge_attrs, edge_feats,
        W_up_s, W_up_v, W1, W2, W3, W4, W_out_s, W_out_v,
    )

    key = (F, NT)
    if key not in _PROG_CACHE:
        _PROG_CACHE[key] = build_program(F, NT)
    nc = _PROG_CACHE[key]

    time_reps = int(os.environ.get("KERNEL_TIME_REPS", "0"))
    profile_dir = os.environ.get("KERNEL_PROFILE_DIR") or None
    results, times = _run_pjrt(
        nc, in_maps, N_CORES, time_reps=time_reps, profile_dir=profile_dir
    )
    if times:
        best = min(times)
        kernel.last_exec_time_ns = int(best * 1e9)
        kernel.last_times = times
        print(f"wall times (s): {[f'{x:.6f}' for x in times]}")

    out_sorted = np.empty((N_EDGES, 4 * MUL), np.float32)
    for c in range(N_CORES):
        ot = np.asarray(results[c]["outT"]).astype(np.float32)[:, :ES]  # [512, ES]
        lo = c * ES
        out_sorted[lo : lo + ES, :MUL] = ot[:MUL].T
        out_sorted[lo : lo + ES, MUL:] = (
            ot[MUL:].reshape(3, MUL, ES).transpose(2, 1, 0).reshape(ES, 3 * MUL)
        )
    out = np.empty_like(out_sorted)
    out[perm] = out_sorted
    return out


# revision 6
# speedup vs baseline: 2.8370x; 1.0974x over previous
"""Trainium2 Bass kernel: e3nn edge message block (gnn_message_passing).

Strategy (edge-parallel across 8 cores, v2):
  - Host: sort edges by sender; each core gets 25000 contiguous sorted
    edges whose senders span a <=2048-node window -> per-core node table
    is 16 blocks instead of 79. Norm constants folded into weights.
  - Device phase A: local node table Tn[p, blk, :] = [s|vx|vy|vz] row of
    window-node (blk*128+p), built with bf16 matmuls.
  - Device phase B, per 512-edge tile (feature-major [128, 512]):
      * gpsimd dma_gather pulls per-edge sender rows from SBUF table
      * y0/y1 broadcasts via ONE partition-broadcast DMA from DRAM
      * radial MLP on PE + ACT silu; uvu tensor product on DVE with
        [128, 3*512] fused ops (stride-0 broadcasts)
      * final linear: 8 accumulating matmuls into two 2-bank PSUM tiles
      * bf16 output, 2 DMAs per tile; host inverse-permutes + upcasts
"""

import os
import sys

sys.path.insert(0, "/opt/trn_rl_repo")

import numpy as np

MUL = 128
N_NODES = 10000
N_EDGES = 200000
N_CORES = 8
ES = N_EDGES // N_CORES          # 25000 edges per core
F = 512                          # edges per tile (free dim)
NT = (ES + F - 1) // F           # 49 tiles
ESP = NT * F                     # 25088 padded edges per core
EDGE_FEAT_DIM = 8
HIDDEN = 64
NWIN = 2048                      # per-core node window (16 blocks of 128)
NBLK = NWIN // 128


def _silu_cst():
    z = np.linspace(-12.0, 12.0, 200001)
    pdf = np.exp(-0.5 * z * z) / np.sqrt(2.0 * np.pi)
    silu = z / (1.0 + np.exp(-z))
    trapz = getattr(np, "trapezoid", None) or getattr(np, "trapz")
    return np.float32(1.0 / np.sqrt(trapz(silu * silu * pdf, z)))


def build_program(f=F, nt=NT):
    """Build the SPMD single-core Bass program (same program on all cores)."""
    import concourse.bass as bass
    import concourse.bacc as bacc
    import concourse.tile as tile
    from concourse import mybir

    f32 = mybir.dt.float32
    bf16 = mybir.dt.bfloat16
    i16 = mybir.dt.int16
    AF = mybir.ActivationFunctionType

    esp = nt * f
    nc = bacc.Bacc(None, target_bir_lowering=False, debug=False)

    # ---- DRAM parameters --------------------------------------------------
    nfT = nc.declare_dram_parameter("nfT", [4 * MUL, NWIN], bf16, isOutput=False)
    idx_d = nc.declare_dram_parameter("idx", [128, nt * (f // 16)], i16, isOutput=False)
    efT_d = nc.declare_dram_parameter("efT", [EDGE_FEAT_DIM, esp], bf16, isOutput=False)
    yT_d = nc.declare_dram_parameter("yT", [1, 4 * esp], bf16, isOutput=False)
    W1_d = nc.declare_dram_parameter("W1", [EDGE_FEAT_DIM, HIDDEN], bf16, isOutput=False)
    W2_d = nc.declare_dram_parameter("W2", [HIDDEN, HIDDEN], bf16, isOutput=False)
    W3_d = nc.declare_dram_parameter("W3", [HIDDEN, HIDDEN], bf16, isOutput=False)
    W4_d = nc.declare_dram_parameter("W4", [HIDDEN, 4 * MUL], bf16, isOutput=False)
    WupS_d = nc.declare_dram_parameter("WupS", [MUL, MUL], bf16, isOutput=False)
    WupV_d = nc.declare_dram_parameter("WupV", [MUL, MUL], bf16, isOutput=False)
    Wout_d = nc.declare_dram_parameter("Wout", [MUL, 4 * MUL], bf16, isOutput=False)
    outT_d = nc.declare_dram_parameter("outT", [4 * MUL, esp], bf16, isOutput=True)

    with tile.TileContext(nc) as tc:
        with (
            tc.tile_pool(name="const", bufs=1) as const,
            tc.tile_pool(name="tables", bufs=1) as tabs,
            tc.tile_pool(name="work", bufs=2) as work,
            tc.tile_pool(name="psum", bufs=2, space="PSUM") as psum,
        ):
            # ---- constants into SBUF -------------------------------------
            def cload(dram, shape, dtype, name):
                t = const.tile(shape, dtype, name=name, tag=name)
                nc.sync.dma_start(out=t[:], in_=dram[:])
                return t

            W1_s = cload(W1_d, [EDGE_FEAT_DIM, HIDDEN], bf16, "cW1")
            W2_s = cload(W2_d, [HIDDEN, HIDDEN], bf16, "cW2")
            W3_s = cload(W3_d, [HIDDEN, HIDDEN], bf16, "cW3")
            W4_s = cload(W4_d, [HIDDEN, 4 * MUL], bf16, "cW4")
            WupS_s = cload(WupS_d, [MUL, MUL], bf16, "cWupS")
            WupV_s = cload(WupV_d, [MUL, MUL], bf16, "cWupV")
            Wout_s = cload(Wout_d, [MUL, 4 * MUL], bf16, "cWout")  # A|B|C|D
            idx_s = const.tile([128, nt * (f // 16)], i16, name="cidx", tag="cidx")
            nc.sync.dma_start(out=idx_s[:], in_=idx_d[:])

            A_s = Wout_s[:, 0:MUL]
            B_s = Wout_s[:, MUL : 2 * MUL]
            C_s = Wout_s[:, 2 * MUL : 3 * MUL]
            D_s = Wout_s[:, 3 * MUL : 4 * MUL]

            # ---- phase A: node-major table (local window) ----------------
            # Tn[p, blk, :] = [s | vx | vy | vz] row of node (blk*128 + p)
            Tn = tabs.tile([128, NBLK, 4 * MUL], bf16)

            wcfg = [WupS_s, WupV_s, WupV_s, WupV_s]
            for c in range(NBLK):
                n0 = c * 128
                pp = psum.tile([128, 4 * MUL], f32, tag="pw", bufs=4)
                xa = work.tile([128, 4, 128], bf16, tag="nfc", bufs=2)
                # one DMA: xa[p, k, :] = nfT[128k + p, n0:n0+128]
                nc.sync.dma_start(
                    out=xa[:],
                    in_=nfT.rearrange("(k p) n -> p k n", k=4)[:, :, n0 : n0 + 128],
                )
                for k in range(4):
                    nc.tensor.matmul(
                        pp[:, 128 * k : 128 * (k + 1)],
                        lhsT=xa[:, k, :],
                        rhs=wcfg[k][:],
                        start=True,
                        stop=True,
                    )
                nc.scalar.activation(Tn[:, c, :], pp[:], AF.Copy)

            # ---- edge radial features: preload all tiles at once ---------
            etA = tabs.tile([EDGE_FEAT_DIM, esp], bf16)
            nc.sync.dma_start(out=etA[:], in_=efT_d[:])

            # ---- phase B: edge tiles -------------------------------------
            for t in range(nt):
                e0 = t * f
                c0 = t * (f // 16)

                # gather sender rows: G1[p, k, i] = row(idx_i)[k*128 + p]
                G1 = work.tile([128, 4, f], bf16, tag="G1", bufs=3)
                nc.gpsimd.dma_gather(
                    G1[:],
                    Tn[:],
                    idx_s[:, c0 : c0 + f // 16],
                    num_idxs=f,
                    num_idxs_reg=f,
                    elem_size=4 * MUL,
                    transpose=True,
                    sbuf_tokens_per_rank=128,
                    sbuf_free_dim_per_rank=4 * MUL * 2,
                    sbuf_free_dim_pad_per_rank=0,
                    sbuf_byte_offset=0,
                )
                s1 = G1[:, 0, :]
                v3 = G1[:, 1:4, :]

                et = etA[:, e0 : e0 + f]

                # all 4 per-edge scalars broadcast in one DMA:
                # yb[p, :] = [y0 | y1x | y1y | y1z] (f each)
                yb = work.tile([128, 4 * f], bf16, tag="yb", bufs=3)
                nc.sync.dma_start(
                    out=yb[:],
                    in_=yT_d[0:1, 4 * e0 : 4 * (e0 + f)].partition_broadcast(128),
                )
                y1b3 = yb[:, f : 4 * f].rearrange("p (m f) -> p m f", m=3)

                # radial MLP (feature-major; silu const folded into W2..W4)
                ph1 = psum.tile([128, f], f32, tag="pw", bufs=4)
                nc.tensor.matmul(ph1[0:HIDDEN, :], lhsT=W1_s[:], rhs=et,
                                 start=True, stop=True)
                h1 = work.tile([HIDDEN, f], bf16, tag="h1", bufs=2)
                nc.scalar.activation(h1[:], ph1[0:HIDDEN, :], AF.Silu)
                ph2 = psum.tile([128, f], f32, tag="pw", bufs=4)
                nc.tensor.matmul(ph2[0:HIDDEN, :], lhsT=W2_s[:], rhs=h1[:],
                                 start=True, stop=True)
                h2 = work.tile([HIDDEN, f], bf16, tag="h2", bufs=2)
                nc.scalar.activation(h2[:], ph2[0:HIDDEN, :], AF.Silu)
                ph3 = psum.tile([128, f], f32, tag="pw", bufs=4)
                nc.tensor.matmul(ph3[0:HIDDEN, :], lhsT=W3_s[:], rhs=h2[:],
                                 start=True, stop=True)
                h3 = work.tile([HIDDEN, f], bf16, tag="h3", bufs=2)
                nc.scalar.activation(h3[:], ph3[0:HIDDEN, :], AF.Silu)

                # h3 * y0 (folds y0 into the w_a and w_d tensor-product paths)
                h3y0 = work.tile([HIDDEN, f], bf16, tag="h3y0", bufs=2)
                nc.vector.tensor_mul(out=h3y0[:], in0=h3[:], in1=yb[0:HIDDEN, 0:f])

                # tpw chunks: a,d use h3*y0 (pre-scaled); b,c use h3
                pwa = psum.tile([128, f], f32, tag="pw", bufs=4)
                nc.tensor.matmul(pwa[:], lhsT=W4_s[:, 0:128], rhs=h3y0[:],
                                 start=True, stop=True)
                pwd = psum.tile([128, f], f32, tag="pw", bufs=4)
                nc.tensor.matmul(pwd[:], lhsT=W4_s[:, 384:512], rhs=h3y0[:],
                                 start=True, stop=True)
                pwb = psum.tile([128, f], f32, tag="pw", bufs=4)
                nc.tensor.matmul(pwb[:], lhsT=W4_s[:, 128:256], rhs=h3[:],
                                 start=True, stop=True)
                pwc = psum.tile([128, f], f32, tag="pw", bufs=4)
                nc.tensor.matmul(pwc[:], lhsT=W4_s[:, 256:384], rhs=h3[:],
                                 start=True, stop=True)

                wdy0 = work.tile([128, f], bf16, tag="wdy0", bufs=2)
                nc.scalar.activation(wdy0[:], pwd[:], AF.Copy)

                # dot = sum_m v1m * y1m
                D3 = work.tile([128, 3 * f], bf16, tag="D3", bufs=2)
                nc.vector.tensor_mul(
                    out=D3[:].rearrange("p (m f) -> p m f", m=3),
                    in0=v3, in1=y1b3,
                )
                dsum = work.tile([128, f], bf16, tag="dsum", bufs=2)
                nc.vector.tensor_add(out=dsum[:], in0=D3[:, 0:f], in1=D3[:, f : 2 * f])
                dot = work.tile([128, f], bf16, tag="dot", bufs=2)
                nc.vector.tensor_add(out=dot[:], in0=dsum[:], in1=D3[:, 2 * f : 3 * f])

                pprime = work.tile([128, f], bf16, tag="pp", bufs=2)
                nc.vector.tensor_mul(out=pprime[:], in0=pwa[:], in1=s1)
                rbar = work.tile([128, f], bf16, tag="rbar", bufs=2)
                nc.vector.tensor_mul(out=rbar[:], in0=pwb[:], in1=dot[:])
                zt = work.tile([128, f], bf16, tag="zt", bufs=2)
                nc.vector.tensor_mul(out=zt[:], in0=pwc[:], in1=s1)

                # q3[p, m, e] = zt * y1m ;  t3[p, m, e] = wdy0 * v1m
                q3 = work.tile([128, 3 * f], bf16, tag="q3", bufs=2)
                nc.vector.tensor_mul(
                    out=q3[:].rearrange("p (m f) -> p m f", m=3),
                    in0=zt[:].unsqueeze(1).broadcast_to([128, 3, f]),
                    in1=y1b3,
                )
                t3 = work.tile([128, 3 * f], bf16, tag="t3", bufs=2)
                nc.vector.tensor_mul(
                    out=t3[:].rearrange("p (m f) -> p m f", m=3),
                    in0=wdy0[:].unsqueeze(1).broadcast_to([128, 3, f]),
                    in1=v3,
                )

                # final linear: scalars into oS, all 3 vector planes in two
                # free=1536 matmuls into the 3-bank oV tile
                oS = psum.tile([128, f], f32, tag="oS", bufs=1)
                nc.tensor.matmul(oS[:], lhsT=A_s, rhs=pprime[:],
                                 start=True, stop=False)
                nc.tensor.matmul(oS[:], lhsT=B_s, rhs=rbar[:],
                                 start=False, stop=True)
                oV = psum.tile([128, 3 * f], f32, tag="oV", bufs=1)
                for m in range(3):
                    sl = slice(m * f, (m + 1) * f)
                    nc.tensor.matmul(oV[:, sl], lhsT=C_s, rhs=q3[:, sl],
                                     start=True, stop=False)
                    nc.tensor.matmul(oV[:, sl], lhsT=D_s, rhs=t3[:, sl],
                                     start=False, stop=True)

                sbO = work.tile([128, 4 * f], bf16, tag="sbO", bufs=3)
                nc.scalar.activation(sbO[:, 0:f], oS[:], AF.Copy)
                nc.scalar.activation(sbO[:, f : 4 * f], oV[:], AF.Copy)

                od = outT_d.rearrange("(c p) e -> p c e", c=4)
                nc.sync.dma_start(
                    out=od[:, :, e0 : e0 + f],
                    in_=sbO[:].rearrange("p (c f) -> p c f", c=4),
                )

    nc.compile()
    return nc


def prep_host_inputs(node_feats, edge_index, edge_attrs, edge_feats,
                     W_up_s, W_up_v, W1, W2, W3, W4, W_out_s, W_out_v,
                     f=F, nt=NT, n_cores=N_CORES):
    """Sort edges, fold constants, build device layouts. Returns (in_maps, perm)."""
    import ml_dtypes

    cst = _silu_cst()
    node_feats = np.asarray(node_feats, dtype=np.float32)
    edge_attrs = np.asarray(edge_attrs, dtype=np.float32)
    edge_feats = np.asarray(edge_feats, dtype=np.float32)
    sender = np.asarray(edge_index)[0].astype(np.int64)

    # sort edges by sender: each core gets a contiguous node window
    perm = np.argsort(sender, kind="stable")
    sender_s = sender[perm]
    edge_attrs = edge_attrs[perm]
    edge_feats = edge_feats[perm]

    esp = nt * f
    n_edges = sender.shape[0]
    es = n_edges // n_cores

    # weights with all norm constants folded
    W1h = (np.asarray(W1, np.float32) / np.sqrt(np.float32(EDGE_FEAT_DIM)))
    W2h = (np.asarray(W2, np.float32) / np.sqrt(np.float32(HIDDEN))) * cst
    W3h = (np.asarray(W3, np.float32) / np.sqrt(np.float32(HIDDEN))) * cst
    W4h = (np.asarray(W4, np.float32) / np.sqrt(np.float32(HIDDEN))) * cst
    inv_sqrt_mul = np.float32(1.0 / np.sqrt(MUL))
    WupSh = np.asarray(W_up_s, np.float32) * inv_sqrt_mul
    WupVh = np.asarray(W_up_v, np.float32) * inv_sqrt_mul
    inv2 = np.float32(1.0 / np.sqrt(2 * MUL))
    A = np.asarray(W_out_s, np.float32)[:MUL] * inv2
    B = np.asarray(W_out_s, np.float32)[MUL:] * (inv2 / np.sqrt(np.float32(3.0)))
    C = np.asarray(W_out_v, np.float32)[:MUL] * inv2
    D = np.asarray(W_out_v, np.float32)[MUL:] * inv2
    Wout = np.concatenate([A, B, C, D], axis=1).astype(ml_dtypes.bfloat16)

    # node features, feature-major planes: s, vx, vy, vz
    nfT = np.empty((4, MUL, N_NODES), np.float32)
    nfT[0] = node_feats[:, :MUL].T
    for m in range(3):
        nfT[1 + m] = node_feats[:, MUL + m :: 3].T
    nfT = np.ascontiguousarray(nfT.reshape(4 * MUL, N_NODES))

    bf = ml_dtypes.bfloat16
    shared = {
        "W1": np.ascontiguousarray(W1h.astype(bf)),
        "W2": np.ascontiguousarray(W2h.astype(bf)),
        "W3": np.ascontiguousarray(W3h.astype(bf)),
        "W4": np.ascontiguousarray(W4h.astype(bf)),
        "WupS": np.ascontiguousarray(WupSh.astype(bf)),
        "WupV": np.ascontiguousarray(WupVh.astype(bf)),
        "Wout": np.ascontiguousarray(Wout),
    }

    in_maps = []
    for c in range(n_cores):
        lo, hi = c * es, (c + 1) * es
        snd = sender_s[lo:hi]
        base = int(snd[0])
        width = int(snd[-1]) - base + 1
        assert width <= NWIN, f"core {c}: node window {width} > {NWIN}"
        base = min(base, N_NODES - NWIN)

        nf_slice = np.zeros((4 * MUL, NWIN), np.float32)
        nf_slice[:, : min(NWIN, N_NODES - base)] = nfT[:, base : base + NWIN]

        loc = np.zeros(esp, np.int16)
        loc[:es] = (snd - base).astype(np.int16)
        # gather layout: idx[16g+p, t*(f//16)+s] = loc[t*f + s*16 + p]
        sp = loc.reshape(nt, f // 16, 16)           # [t, s, p]
        grid16 = sp.transpose(2, 0, 1).reshape(16, nt * (f // 16))
        idx_l = np.ascontiguousarray(np.tile(grid16, (8, 1)))

        efT = np.zeros((EDGE_FEAT_DIM, esp), np.float32)
        efT[:, :es] = edge_feats[lo:hi].T
        efT = efT.astype(bf)
        yT = np.zeros((4, esp), np.float32)
        yT[:, :es] = edge_attrs[lo:hi].T
        # per-tile flat layout: [1, t*4f + r*f + e]
        y_flat = np.ascontiguousarray(
            yT.reshape(4, nt, f).transpose(1, 0, 2).reshape(1, 4 * esp)
        ).astype(bf)

        in_maps.append(dict(shared, nfT=nf_slice.astype(bf), idx=idx_l,
                            efT=efT, yT=y_flat))
    return in_maps, perm


_PROG_CACHE = {}


def _run_pjrt(nc, in_maps, n_cores=N_CORES, time_reps=0, profile_dir=None):
    """Execute the SPMD program via PJRT. Returns (results, wall_times)."""
    import time as _time

    import jax
    from jax.sharding import Mesh, NamedSharding, PartitionSpec

    try:
        from jax.experimental.shard_map import shard_map
    except ImportError:  # newer jax
        from jax.sharding import shard_map
    from concourse import bass2jax, mybir

    bass2jax.install_neuronx_cc_hook()

    save_neff = os.environ.get("KERNEL_SAVE_NEFF")
    if save_neff:
        _orig_rename = bass2jax.rename_neff_tensors_and_patch_header.__wrapped__ if hasattr(
            bass2jax.rename_neff_tensors_and_patch_header, "__wrapped__"
        ) else bass2jax.rename_neff_tensors_and_patch_header

        def _rename_and_save(neff_file, renames):
            data = _orig_rename(neff_file, renames)
            with open(save_neff, "wb") as fh:
                fh.write(data)
            return data

        bass2jax.rename_neff_tensors_and_patch_header = _rename_and_save

    partition_name = (
        nc.partition_id_tensor.name if nc.partition_id_tensor is not None else None
    )
    in_names, out_names, out_avals, zero_outs = [], [], [], []
    for alloc in nc.m.functions[0].allocations:
        if not isinstance(alloc, mybir.MemoryLocationSet):
            continue
        name = alloc.memorylocations[0].name
        if alloc.kind == "ExternalInput":
            if name != partition_name:
                in_names.append(name)
        elif alloc.kind == "ExternalOutput":
            shape = tuple(alloc.tensor_shape)
            dtype = mybir.dt.np(alloc.dtype)
            out_names.append(name)
            out_avals.append(jax.core.ShapedArray(shape, dtype))
            zero_outs.append(np.zeros(shape, dtype))
    n_params = len(in_names)
    in_names_all = in_names + out_names
    if partition_name is not None:
        in_names_all = in_names_all + [partition_name]

    def _body(*args):
        operands = list(args)
        if partition_name is not None:
            operands.append(bass2jax.partition_id_tensor())
        outs = bass2jax._bass_exec_p.bind(
            *operands,
            out_avals=tuple(out_avals),
            in_names=tuple(in_names_all),
            out_names=tuple(out_names),
            lowering_input_output_aliases=(),
            sim_require_finite=True,
            sim_require_nnan=True,
            nc=nc,
        )
        return tuple(outs)

    devices = jax.devices()[:n_cores]
    mesh = Mesh(np.asarray(devices), ("core",))
    nouts = len(out_names)
    donate = tuple(range(n_params, n_params + nouts))
    sharded = jax.jit(
        shard_map(
            _body,
            mesh=mesh,
            in_specs=(PartitionSpec("core"),) * (n_params + nouts),
            out_specs=(PartitionSpec("core"),) * nouts,
            check_rep=False,
        ),
        donate_argnums=donate,
        keep_unused=True,
    )

    spec = NamedSharding(mesh, PartitionSpec("core"))
    dev_in = [
        jax.device_put(
            np.concatenate([np.asarray(in_maps[c][nm]) for c in range(n_cores)], axis=0),
            spec,
        )
        for nm in in_names
    ]

    def make_zeros():
        return [
            jax.device_put(np.zeros((n_cores * z.shape[0], *z.shape[1:]), z.dtype), spec)
            for z in zero_outs
        ]

    out_arrs = jax.block_until_ready(sharded(*dev_in, *make_zeros()))

    times = []
    prof_ctx = None
    if profile_dir:
        prof_ctx = _ntff_profiler()
    for r in range(max(time_reps, 0)):
        zs = make_zeros()
        jax.block_until_ready(zs)
        do_prof = prof_ctx is not None and r == time_reps - 1
        if do_prof:
            prof_ctx.start()
        t0 = _time.perf_counter()
        out_arrs = jax.block_until_ready(sharded(*dev_in, *zs))
        times.append(_time.perf_counter() - t0)
        if do_prof:
            prof_ctx.stop(profile_dir)

    results = [
        {
            nm: np.asarray(out_arrs[i]).reshape(n_cores, *out_avals[i].shape)[c]
            for i, nm in enumerate(out_names)
        }
        for c in range(n_cores)
    ]
    return results, times


class _ntff_profiler:
    def __init__(self, so_path="/opt/axon/libaxon_pjrt.so"):
        import ctypes

        self.lib = ctypes.CDLL(so_path)
        self.ctypes = ctypes
        self.lib.axon_start_nrt_profile.argtypes = [
            ctypes.POINTER(ctypes.c_int64),
            ctypes.c_size_t,
        ]
        self.lib.axon_start_nrt_profile.restype = ctypes.c_int64
        self.lib.axon_stop_nrt_profile.argtypes = [ctypes.c_char_p]
        self.lib.axon_stop_nrt_profile.restype = ctypes.c_int64

    def start(self):
        rc = self.lib.axon_start_nrt_profile(None, 0)
        if rc != 0:
            print(f"ntff profile start failed rc={rc}")

    def stop(self, outdir):
        os.makedirs(outdir, exist_ok=True)
        n = self.lib.axon_stop_nrt_profile(str(outdir).encode())
        print(f"ntff profile: {n} file(s) -> {outdir}")


def kernel(node_feats, edge_index, edge_attrs, edge_feats,
           W_up_s, W_up_v, W1, W2, W3, W4, W_out_s, W_out_v):
    in_maps, perm = prep_host_inputs(
        node_feats, edge_index, edge_attrs, edge_feats,
        W_up_s, W_up_v, W1, W2, W3, W4, W_out_s, W_out_v,
    )

    key = (F, NT)
    if key not in _PROG_CACHE:
        _PROG_CACHE[key] = build_program(F, NT)
    nc = _PROG_CACHE[key]

    time_reps = int(os.environ.get("KERNEL_TIME_REPS", "0"))
    profile_dir = os.environ.get("KERNEL_PROFILE_DIR") or None
    results, times = _run_pjrt(
        nc, in_maps, N_CORES, time_reps=time_reps, profile_dir=profile_dir
    )
    if times:
        best = min(times)
        kernel.last_exec_time_ns = int(best * 1e9)
        kernel.last_times = times
        print(f"wall times (s): {[f'{x:.6f}' for x in times]}")

    out_sorted = np.empty((N_EDGES, 4 * MUL), np.float32)
    for c in range(N_CORES):
        ot = np.asarray(results[c]["outT"]).astype(np.float32)[:, :ES]  # [512, ES]
        lo = c * ES
        out_sorted[lo : lo + ES, :MUL] = ot[:MUL].T
        out_sorted[lo : lo + ES, MUL:] = (
            ot[MUL:].reshape(3, MUL, ES).transpose(2, 1, 0).reshape(ES, 3 * MUL)
        )
    out = np.empty_like(out_sorted)
    out[perm] = out_sorted
    return out
